# revision 2
# baseline (speedup 1.0000x reference)
"""Trainium2 Bass kernel for nn_EncoderOnlyBlock (4-head full-dim encoder block).

Sharding: fully data-parallel, no collectives. 8 cores = (batch b, seq-half).
Each core computes its 1024 query tokens end-to-end for all 4 heads; K/V work
for the full 2048-token batch row is recomputed on both cores of a batch
(the only duplicated work).

Per-core math (all matmuls bf16 inputs, fp32 PSUM accumulation):
  x_perm = [own-half tokens; other-half tokens]           (host permute)
  Q^T_h = Wq_h^T @ x_perm^T[:, :1024] + bq_h              [e, si]
  K^T_h = Wk_h^T @ x_perm^T                               [e, sj]   (bk dropped:
          rows of softmax are invariant to the q·bk and bq·bk terms)
  S     = Q^T^T K^T / sqrt(D); A = exp(S) / rowsum        (no max-sub: |S|<2)
  M_h   = x_perm^T @ A^T                                  [d, si]
  hd^T_h= Wv_h^T @ M_h                                    [e, si]   (== (A@V)^T,
          bv_h folded into cvec since rows of A sum to 1)
  proj  = sum_h hd_h @ W1_h + cvec,  cvec = b1 + sum_h bv_h @ W1_h  (host)
  y     = LN(x_res + proj) * g1 + be1
  out   = LN(y + y @ W2 + b2) * g2 + be2
"""

import numpy as np
import ml_dtypes

BF = ml_dtypes.bfloat16
P = 128
D = 1024
S = 2048
SI = 1024
H = 4
ET = D // P       # 8 e/d/f 128-blocks
SJT = S // P      # 16 sj 128-blocks
SIT = SI // P     # 8 si 128-blocks
SCALE = 1.0 / 32.0  # 1/sqrt(D)
EPS = 1e-5

_CACHE = {}


def _emit(nc, tc, A):
    """Emit the per-core program. A: dict name -> dram AP."""
    from contextlib import ExitStack

    import concourse.bass as bass
    import concourse.mybir as mybir
    from concourse.masks import make_identity

    f32 = mybir.dt.float32
    bf16 = mybir.dt.bfloat16
    Act = mybir.ActivationFunctionType
    Alu = mybir.AluOpType

    with ExitStack() as ctx:
        consts = ctx.enter_context(tc.tile_pool(name="consts", bufs=1))
        psA = ctx.enter_context(tc.tile_pool(name="psA", bufs=3, space="PSUM"))
        psB = ctx.enter_context(tc.tile_pool(name="psB", bufs=2, space="PSUM"))

        ident = consts.tile([P, P], bf16, tag="ident")
        make_identity(nc, ident[:])
        bqr_sb = consts.tile([P, H * ET], f32, tag="bqr")
        nc.sync.dma_start(out=bqr_sb[:], in_=A["bqr"][:])
        cvec_sb = consts.tile([1, D], bf16, tag="cvec")
        nc.sync.dma_start(out=cvec_sb[:], in_=A["cvec"][:])
        b2v_sb = consts.tile([1, D], bf16, tag="b2v")
        nc.sync.dma_start(out=b2v_sb[:], in_=A["b2v"][:])
        ones_sb = consts.tile([1, P], bf16, tag="ones")
        nc.vector.memset(ones_sb[:], 1.0)
        eps_sb = consts.tile([P, 1], f32, tag="eps")
        nc.vector.memset(eps_sb[:], EPS)

        head_ctx = ExitStack()
        xpool = head_ctx.enter_context(tc.tile_pool(name="xp", bufs=1))
        wqkv_pool = head_ctx.enter_context(tc.tile_pool(name="wqkv", bufs=3))
        w1_pool = head_ctx.enter_context(tc.tile_pool(name="w1", bufs=8))
        qt_pool = head_ctx.enter_context(tc.tile_pool(name="qt", bufs=1))
        kt_pool = head_ctx.enter_context(tc.tile_pool(name="kt", bufs=1))
        attn_pool = head_ctx.enter_context(tc.tile_pool(name="at", bufs=2))
        atT_pool = head_ctx.enter_context(tc.tile_pool(name="atT", bufs=1))
        m_pool = head_ctx.enter_context(tc.tile_pool(name="m", bufs=1))
        ht_pool = head_ctx.enter_context(tc.tile_pool(name="ht", bufs=1))
        proj_pool = head_ctx.enter_context(tc.tile_pool(name="pj", bufs=1))
        red_pool = head_ctx.enter_context(tc.tile_pool(name="red", bufs=8))

        xt_sb = xpool.tile([P, ET, S], bf16, tag="xt")
        for c in range(ET):
            nc.sync.dma_start(out=xt_sb[:, c, :], in_=A["xt"][c * P:(c + 1) * P, :])
        xn_sb = xpool.tile([P, SJT, D], bf16, tag="xn")
        for j in range(SJT):
            nc.sync.dma_start(out=xn_sb[:, j, :], in_=A["xn"][j * P:(j + 1) * P, :])

        proj_sb = proj_pool.tile([P, SIT, D], bf16, tag="proj")

        for h in range(H):
            # ---- K^T = Wk^T @ x^T : [e, sj]
            kt_sb = kt_pool.tile([P, ET, S], bf16, tag="kt")
            for c in range(ET):
                wk_c = wqkv_pool.tile([P, ET, P], bf16, tag="wqkv")
                nc.sync.dma_start(out=wk_c[:], in_=A["wkb"][h, c])
                for hs in range(2):
                    ps = psA.tile([P, 1024], f32, tag="psA")
                    for nb in range(2):
                        for kc in range(ET):
                            nc.tensor.matmul(
                                ps[:, nb * 512:(nb + 1) * 512],
                                lhsT=wk_c[:, kc, :],
                                rhs=xt_sb[:, kc, hs * 1024 + nb * 512:hs * 1024 + (nb + 1) * 512],
                                start=(kc == 0), stop=(kc == ET - 1),
                            )
                    nc.vector.tensor_copy(kt_sb[:, c, hs * 1024:(hs + 1) * 1024], ps[:])

            # ---- Q^T = Wq^T @ x^T[:, :1024] + bq : [e, si]
            qt_sb = qt_pool.tile([P, ET, SI], bf16, tag="qt")
            for c in range(ET):
                wq_c = wqkv_pool.tile([P, ET, P], bf16, tag="wqkv")
                nc.sync.dma_start(out=wq_c[:], in_=A["wqb"][h, c])
                ps = psA.tile([P, 1024], f32, tag="psA")
                for nb in range(2):
                    for kc in range(ET):
                        nc.tensor.matmul(
                            ps[:, nb * 512:(nb + 1) * 512],
                            lhsT=wq_c[:, kc, :],
                            rhs=xt_sb[:, kc, nb * 512:(nb + 1) * 512],
                            start=(kc == 0), stop=(kc == ET - 1),
                        )
                nc.scalar.activation(
                    out=qt_sb[:, c, :], in_=ps[:], func=Act.Identity,
                    bias=bqr_sb[:, h * ET + c:h * ET + c + 1],
                )

            # ---- attention, per si-quarter (256 query tokens)
            m_sb = m_pool.tile([P, ET, SI], bf16, tag="m")
            for q in range(4):
                at_sb = atT_pool.tile([P, SJT, 256], bf16, tag="atT")
                attn_t = []
                for t2 in range(2):
                    t = q * 2 + t2
                    a_t = attn_pool.tile([P, S], bf16, tag="attn")
                    attn_t.append(a_t)
                    r = red_pool.tile([P, 2], f32, tag="rsum")
                    rec = red_pool.tile([P, 1], f32, tag="rec")
                    for hs in range(2):
                        ps = psA.tile([P, 1024], f32, tag="psA")
                        for nb in range(2):
                            for kc in range(ET):
                                nc.tensor.matmul(
                                    ps[:, nb * 512:(nb + 1) * 512],
                                    lhsT=qt_sb[:, kc, t * P:(t + 1) * P],
                                    rhs=kt_sb[:, kc, hs * 1024 + nb * 512:hs * 1024 + (nb + 1) * 512],
                                    start=(kc == 0), stop=(kc == ET - 1),
                                )
                        nc.scalar.activation(
                            out=a_t[:, hs * 1024:(hs + 1) * 1024], in_=ps[:],
                            func=Act.Exp, scale=SCALE,
                            accum_out=r[:, hs:hs + 1],
                        )
                    nc.vector.tensor_add(rec[:], r[:, 0:1], r[:, 1:2])
                    nc.vector.reciprocal(rec[:], rec[:])
                    nc.vector.tensor_scalar_mul(a_t[:], a_t[:], rec[:])
                # transpose 2 si-blocks x 16 sj-blocks, batched 4 sj-blocks/copy
                for j4 in range(4):
                    pb = psB.tile([P, 1024], bf16, tag="psB")
                    for jj in range(4):
                        j = j4 * 4 + jj
                        for t2 in range(2):
                            nc.tensor.transpose(
                                pb[:, jj * 256 + t2 * P:jj * 256 + (t2 + 1) * P],
                                attn_t[t2][:, j * P:(j + 1) * P],
                                ident[:],
                            )
                    nc.vector.tensor_copy(at_sb[:, j4 * 4:(j4 + 1) * 4, :], pb[:])
                # M = x^T @ A^T for this quarter : [d, 256]
                for dc in range(ET):
                    ps = psA.tile([P, 1024], f32, tag="psA")
                    for j in range(SJT):
                        nc.tensor.matmul(
                            ps[:, 0:256],
                            lhsT=xn_sb[:, j, dc * P:(dc + 1) * P],
                            rhs=at_sb[:, j, :],
                            start=(j == 0), stop=(j == SJT - 1),
                        )
                    nc.vector.tensor_copy(m_sb[:, dc, q * 256:(q + 1) * 256], ps[:, 0:256])

            # ---- head^T = Wv^T @ M : [e, si]
            ht_sb = ht_pool.tile([P, ET, SI], bf16, tag="ht")
            for eb in range(ET):
                wv_eb = wqkv_pool.tile([P, ET, P], bf16, tag="wqkv")
                nc.sync.dma_start(out=wv_eb[:], in_=A["wvb"][h, eb])
                ps = psA.tile([P, 1024], f32, tag="psA")
                for nb in range(2):
                    for kc in range(ET):
                        nc.tensor.matmul(
                            ps[:, nb * 512:(nb + 1) * 512],
                            lhsT=wv_eb[:, kc, :],
                            rhs=m_sb[:, kc, nb * 512:(nb + 1) * 512],
                            start=(kc == 0), stop=(kc == ET - 1),
                        )
                nc.vector.tensor_copy(ht_sb[:, eb, :], ps[:])

            # ---- proj += head_h @ W1_h (+ cvec once)
            w1_tiles = []
            for eb in range(ET):
                w1_eb = w1_pool.tile([P, D], bf16, tag="w1")
                nc.sync.dma_start(
                    out=w1_eb[:], in_=A["w1"][(h * ET + eb) * P:(h * ET + eb + 1) * P, :]
                )
                w1_tiles.append(w1_eb)
            for t in range(SIT):
                ps = psA.tile([P, 1024], f32, tag="psA")
                for nb in range(2):
                    for eb in range(ET):
                        nc.tensor.matmul(
                            ps[:, nb * 512:(nb + 1) * 512],
                            lhsT=ht_sb[:, eb, t * P:(t + 1) * P],
                            rhs=w1_tiles[eb][:, nb * 512:(nb + 1) * 512],
                            start=(eb == 0), stop=(eb == ET - 1 and h != 0),
                        )
                    if h == 0:
                        nc.tensor.matmul(
                            ps[:, nb * 512:(nb + 1) * 512],
                            lhsT=ones_sb[:, :],
                            rhs=cvec_sb[:, nb * 512:(nb + 1) * 512],
                            start=False, stop=True,
                        )
                if h == 0:
                    nc.scalar.copy(proj_sb[:, t, :], ps[:])
                else:
                    nc.vector.tensor_add(proj_sb[:, t, :], proj_sb[:, t, :], ps[:])

        head_ctx.close()

        # ================= LN1 -> W2 -> LN2 =================
        with ExitStack() as lctx:
            lnp = lctx.enter_context(tc.tile_pool(name="lnp", bufs=1))
            xr_pool = lctx.enter_context(tc.tile_pool(name="xr", bufs=3))
            ybf_pool = lctx.enter_context(tc.tile_pool(name="ybf", bufs=8))
            w2_pool = lctx.enter_context(tc.tile_pool(name="w2", bufs=8))
            st_pool = lctx.enter_context(tc.tile_pool(name="st", bufs=8))
            t2_pool = lctx.enter_context(tc.tile_pool(name="t2", bufs=3))

            gbe_sb = lnp.tile([P, 4, D], f32, tag="gbe")
            gbe_bc = bass.AP(
                tensor=A["gbe"].tensor, offset=A["gbe"].offset,
                ap=[[0, P], A["gbe"].ap[0], A["gbe"].ap[1]],
            )
            nc.gpsimd.dma_start(out=gbe_sb[:], in_=gbe_bc)
            y_sb = lnp.tile([P, SIT, D], f32, tag="y")
            yt_sb = lnp.tile([P, ET, SI], bf16, tag="yt")

            def layer_norm(dst, src, mv_slot, g_idx):
                """dst = LN(src) * g + be, src/dst [P, D] f32 (may alias)."""
                st = st_pool.tile([P, 2, 6], f32, tag="st")
                nc.vector.bn_stats(st[:, 0, :], src[:, 0:512])
                nc.vector.bn_stats(st[:, 1, :], src[:, 512:1024])
                mv = st_pool.tile([P, 2], f32, tag="mv")
                nc.vector.bn_aggr(mv[:], st[:])
                rstd = st_pool.tile([P, 1], f32, tag="rstd")
                nc.scalar.activation(
                    out=rstd[:], in_=mv[:, 1:2], func=Act.Sqrt, bias=eps_sb[:]
                )
                nc.vector.reciprocal(rstd[:], rstd[:])
                nc.vector.tensor_scalar(
                    dst, src, scalar1=mv[:, 0:1], scalar2=rstd[:],
                    op0=Alu.subtract, op1=Alu.mult,
                )
                nc.vector.tensor_mul(dst, dst, gbe_sb[:, 2 * g_idx, :])
                nc.vector.tensor_add(dst, dst, gbe_sb[:, 2 * g_idx + 1, :])

            ybf_tiles = []
            for t in range(SIT):
                xr = xr_pool.tile([P, D], f32, tag="xr")
                nc.sync.dma_start(out=xr[:], in_=A["xres"][t * P:(t + 1) * P, :])
                t1 = t2_pool.tile([P, D], f32, tag="t1")
                nc.vector.tensor_add(t1[:], xr[:], proj_sb[:, t, :])
                layer_norm(y_sb[:, t, :], t1[:], None, 0)
                yb = ybf_pool.tile([P, D], bf16, tag="ybf")
                nc.scalar.copy(yb[:], y_sb[:, t, :])
                ybf_tiles.append(yb)

            for fb in range(ET):
                pb = psB.tile([P, 1024], bf16, tag="psB")
                for t in range(SIT):
                    nc.tensor.transpose(
                        pb[:, t * P:(t + 1) * P],
                        ybf_tiles[t][:, fb * P:(fb + 1) * P],
                        ident[:],
                    )
                nc.vector.tensor_copy(yt_sb[:, fb, :], pb[:])

            w2_tiles = []
            for kc in range(ET):
                w2_kc = w2_pool.tile([P, D], bf16, tag="w2")
                nc.sync.dma_start(out=w2_kc[:], in_=A["w2"][kc * P:(kc + 1) * P, :])
                w2_tiles.append(w2_kc)

            for t in range(SIT):
                ps = psA.tile([P, 1024], f32, tag="psA")
                for nb in range(2):
                    for kc in range(ET):
                        nc.tensor.matmul(
                            ps[:, nb * 512:(nb + 1) * 512],
                            lhsT=yt_sb[:, kc, t * P:(t + 1) * P],
                            rhs=w2_tiles[kc][:, nb * 512:(nb + 1) * 512],
                            start=(kc == 0), stop=False,
                        )
                    nc.tensor.matmul(
                        ps[:, nb * 512:(nb + 1) * 512],
                        lhsT=ones_sb[:, :],
                        rhs=b2v_sb[:, nb * 512:(nb + 1) * 512],
                        start=False, stop=True,
                    )
                t2 = t2_pool.tile([P, D], f32, tag="t2")
                nc.vector.tensor_add(t2[:], y_sb[:, t, :], ps[:])
                ot = t2_pool.tile([P, D], f32, tag="ot")
                layer_norm(ot[:], t2[:], None, 1)
                nc.sync.dma_start(out=A["out"][t * P:(t + 1) * P, :], in_=ot[:])


def _build():
    import concourse.bass as bass
    import concourse.mybir as mybir
    import concourse.tile as tile
    from concourse import bacc

    f32 = mybir.dt.float32
    bf16 = mybir.dt.bfloat16

    nc = bacc.Bacc("TRN2", target_bir_lowering=False, debug=False, num_devices=8)
    A = {}

    def din(name, shape, dt):
        A[name] = nc.dram_tensor(name, shape, dt, kind="ExternalInput").ap()

    din("xt", [D, S], bf16)
    din("xn", [S, D], bf16)
    din("xres", [SI, D], f32)
    din("wqb", [H, ET, P, ET, P], bf16)
    din("wkb", [H, ET, P, ET, P], bf16)
    din("wvb", [H, ET, P, ET, P], bf16)
    din("w1", [H * D, D], bf16)
    din("w2", [D, D], bf16)
    din("bqr", [P, H * ET], f32)
    din("cvec", [1, D], bf16)
    din("b2v", [1, D], bf16)
    din("gbe", [4, D], f32)
    A["out"] = nc.dram_tensor("out", [SI, D], f32, kind="ExternalOutput").ap()

    with tile.TileContext(nc) as tc:
        _emit(nc, tc, A)
    nc.compile()
    return nc


def _get_nc():
    if "nc" not in _CACHE:
        _CACHE["nc"] = _build()
    return _CACHE["nc"]


def _prep_inputs(inputs):
    x = np.ascontiguousarray(inputs["embedding_matrix"], dtype=np.float32)
    Wq = np.asarray(inputs["Wq"], np.float32)
    bq = np.asarray(inputs["bq"], np.float32)
    Wv = np.asarray(inputs["Wv"], np.float32)
    bv = np.asarray(inputs["bv"], np.float32)
    Wk = np.asarray(inputs["Wk"], np.float32)
    W1 = np.asarray(inputs["W1"], np.float32)
    b1 = np.asarray(inputs["b1"], np.float32)
    W2 = np.asarray(inputs["W2"], np.float32)
    b2 = np.asarray(inputs["b2"], np.float32)
    g1 = np.asarray(inputs["g1"], np.float32)
    be1 = np.asarray(inputs["be1"], np.float32)
    g2 = np.asarray(inputs["g2"], np.float32)
    be2 = np.asarray(inputs["be2"], np.float32)

    def pack_w(W):  # [H, D, D] -> [H, ET, P(row-in-block), ET(kc), P] lhsT blocks
        return np.ascontiguousarray(
            W.reshape(H, ET, P, ET, P).transpose(0, 3, 2, 1, 4).astype(BF)
        )

    wqb = pack_w(Wq)
    wkb = pack_w(Wk)
    wvb = pack_w(Wv)
    w1b = np.ascontiguousarray(W1.astype(BF))
    w2b = np.ascontiguousarray(W2.astype(BF))
    # bq rearranged so bias for (h, e-block c) is column h*ET+c: [P, H*ET]
    bqr = np.ascontiguousarray(bq.reshape(H, ET, P).transpose(2, 0, 1).reshape(P, H * ET))
    cvec = (b1 + sum(bv[h] @ W1[h * D:(h + 1) * D] for h in range(H)))
    cvec = np.ascontiguousarray(cvec.reshape(1, D).astype(BF))
    b2v = np.ascontiguousarray(b2.reshape(1, D).astype(BF))
    gbe = np.ascontiguousarray(np.stack([g1, be1, g2, be2]))

    shared = {
        "wqb": wqb, "wkb": wkb, "wvb": wvb, "w1": w1b, "w2": w2b,
        "bqr": bqr, "cvec": cvec, "b2v": b2v, "gbe": gbe,
    }
    in_maps = []
    for core in range(8):
        b, half = core // 2, core % 2
        own = x[b, half * SI:(half + 1) * SI]
        other = x[b, (1 - half) * SI:(2 - half) * SI]
        xperm = np.concatenate([own, other], axis=0)
        m = dict(shared)
        m["xn"] = np.ascontiguousarray(xperm.astype(BF))
        m["xt"] = np.ascontiguousarray(xperm.T.astype(BF))
        m["xres"] = np.ascontiguousarray(own)
        in_maps.append(m)
    return in_maps


def kernel(**inputs):
    from concourse.bass_utils import run_bass_kernel_spmd

    nc = _get_nc()
    in_maps = _prep_inputs(inputs)
    res = run_bass_kernel_spmd(nc, in_maps, core_ids=list(range(8)))
    out = np.empty((4, S, D), np.float32)
    for core in range(8):
        b, half = core // 2, core % 2
        out[b, half * SI:(half + 1) * SI] = res.results[core]["out"]
    return out
